# revision 28
# baseline (speedup 1.0000x reference)
"""Trainium2 Bass kernel for nn_BoundaryLoss (retrieval 1-NN + boundary loss).

Math: rigid SE(3) transforms preserve distances/dot-products, so 1-NN and the
signed-distance dot product are done in the GLOBAL frame (wg = R_b w + t_b on
host).  The device scores every (waypoint, boundary-point) pair with
    s16[n] = -(d^2)/2 = w.p - |p|^2/2 - |w|^2/2
via a K=12-row fp16 hi/lo split matmul (per-coord [ah*bh + ah*bl + al*bh],
two rows for p^2/2, one bias row for |w|^2/2).  Subtracting the per-waypoint
|w|^2/2 inside the matmul recenters scores near 0 where fp16 spacing is
~0.03-0.12, collapsing argmax ties (validated: 6/6400 flips, loss rel err
7.5e-4).

Per core (8-way data parallel over (b,t); 7 tiles of 128 waypoints):
  - PE: 10x 2048-wide fp16 matmuls -> PSUM fp32 (4 banks per tile, 2 in flight)
  - ACT+DVE: evacuate 40x 512-wide PSUM slices -> fp16 s16 (split ~3:1)
  - DVE: tensor_tensor max fold tree (2x_1p mode) [128,640,32] -> sub-block
    maxima [128,640]; MAX8 + FIND_INDEX8 on 640 give the best sub-block
  - GPSIMD: one 512B-row indirect gather of that sub-block's [pg,p2] x32
  - DVE: exact fp32 rescore of the 32 candidates, eq/select/min index pick
  - GPSIMD: payload gather [ng, pg.ng] by final index
  - DVE/ACT: dots, exp_relu, masked accumulate; PE ones-matmul -> [1,7] sums
Host: input prep + final sum/6400.

HW notes inherited from measurement: engine reads must stay within one PSUM
bank (512 fp32); DMA cannot touch PSUM; MAX8/FIND_INDEX8 run 1x (hence the
2x fold tree); float32r matmul quantizes (hence fp16 splits); indirect DMA
offsets must be single-index [128,1].
"""

import sys

sys.path.insert(0, "/opt/trn_rl_repo")

import numpy as np

from concourse import bacc, bass, mybir
import concourse.tile as tile
from concourse.bass_utils import run_bass_kernel_spmd

B, T, N = 64, 100, 20000
NCORES = 8
WPC = B * T // NCORES          # 800 waypoints per core
NTILES = 7                     # ceil(WPC / 128)
WPAD = NTILES * 128            # 896
SLICE = 512                    # one PSUM bank of fp32
NSLICES = 40
NPAD = NSLICES * SLICE         # 20480
SB = 32                        # sub-block size for the fold tree
NSB = NPAD // SB               # 640
NSB_REAL = N // SB             # 625
MMW = 512                      # matmul width (1 PSUM bank; ISA max)
NMM = NPAD // MMW              # 40
KS = 12                        # contraction rows (9 split + 2 p2 + 1 bias)
DVE_EVERY = 4                  # slice s -> DVE evac when s % DVE_EVERY == 3

F32 = mybir.dt.float32
F16 = mybir.dt.float16
U16 = mybir.dt.uint16
U32 = mybir.dt.uint32
U8 = mybir.dt.uint8
OP = mybir.AluOpType
AX = mybir.AxisListType
AF = mybir.ActivationFunctionType


def build():
    nc = bacc.Bacc("TRN2", target_bir_lowering=False, debug=False,
                   num_devices=NCORES)
    lhs = nc.dram_tensor("lhs", [128, WPAD], F16, kind="ExternalInput").ap()
    rhs = nc.dram_tensor("rhs", [128, NPAD // 4], F16, kind="ExternalInput").ap()
    wgv = nc.dram_tensor("wgv", [128, NTILES, 3], F32, kind="ExternalInput").ap()
    msk = nc.dram_tensor("msk", [128, NTILES], F32, kind="ExternalInput").ap()
    sbt = nc.dram_tensor("sbt", [NSB_REAL, SB * 4], F32, kind="ExternalInput").ap()
    tbl = nc.dram_tensor("tbl", [N, 4], F32, kind="ExternalInput").ap()
    out = nc.dram_tensor("out", [1, NTILES], F32, kind="ExternalOutput").ap()

    with tile.TileContext(nc) as tc:
        with (
            tc.tile_pool(name="const", bufs=1) as cpool,
            tc.tile_pool(name="s16p", bufs=2) as s16p,
            tc.tile_pool(name="fold", bufs=1) as fp,
            tc.tile_pool(name="sb", bufs=3) as sbp,
            tc.tile_pool(name="ps", bufs=7, space="PSUM") as ps,
            tc.tile_pool(name="ps1", bufs=1, space="PSUM") as ps1,
        ):
            lhs_sb = cpool.tile([128, WPAD], F16)
            nc.sync.dma_start(out=lhs_sb[:], in_=lhs[:])
            rhs_sb = cpool.tile([128, NPAD // 4], F16)
            for i in range(4):
                nc.sync.dma_start(out=rhs_sb[32 * i:32 * (i + 1), :],
                                  in_=rhs[32 * i:32 * (i + 1), :])
            wgv_sb = cpool.tile([128, NTILES, 3], F32)
            nc.sync.dma_start(out=wgv_sb[:], in_=wgv[:])
            msk_sb = cpool.tile([128, NTILES], F32)
            nc.sync.dma_start(out=msk_sb[:], in_=msk[:])
            ones_sb = cpool.tile([128, 1], F32)
            nc.vector.memset(ones_sb[:], 1.0)
            er_sb = cpool.tile([128, NTILES], F32)
            nc.vector.memset(er_sb[:], 0.0)

            # ---- software-pipelined stages (gathers get a full tile of
            # slack before their consumers hit the DVE queue head) ----
            st = {}

            def stage_a1(j):
                s16 = s16p.tile([128, NPAD], F16, tag="s16")
                # 4-way PE row-tiling: row group i holds a weight replica
                for c in range(NMM // 4):
                    for i in range(4):
                        s = i * (NMM // 4) + c
                        pg = ps.tile([128, SLICE], F32, tag="mm")
                        nc.tensor.matmul(
                            out=pg[:],
                            lhsT=lhs_sb[32 * i:32 * i + KS,
                                        j * 128:(j + 1) * 128],
                            rhs=rhs_sb[32 * i:32 * i + KS,
                                       c * SLICE:(c + 1) * SLICE],
                            tile_position=(32 * i, 0),
                            start=True, stop=True,
                        )
                        dst = s16[:, s * SLICE:(s + 1) * SLICE]
                        if s % DVE_EVERY == DVE_EVERY - 1 or s == 18:
                            nc.vector.tensor_copy(dst, pg[:])
                        else:
                            nc.scalar.activation(dst, pg[:], AF.Copy)
                st[j] = {"s16": s16}

            def stage_a2(j):
                s16 = st[j].pop("s16")
                # fold tree: [128, 640, 32] -> sub-block maxima [128, 640]
                s16v = s16[:].rearrange("p (a b) -> p a b", b=SB)
                f1 = fp.tile([128, NSB * 16], F16, tag="f1")
                f1v = f1[:].rearrange("p (a b) -> p a b", b=16)
                nc.vector.tensor_tensor(out=f1v, in0=s16v[:, :, 0:16],
                                        in1=s16v[:, :, 16:32], op=OP.max)
                f2 = fp.tile([128, NSB * 8], F16, tag="f2")
                f2v = f2[:].rearrange("p (a b) -> p a b", b=8)
                nc.vector.tensor_tensor(out=f2v, in0=f1v[:, :, 0:8],
                                        in1=f1v[:, :, 8:16], op=OP.max)
                f3 = fp.tile([128, NSB * 4], F16, tag="f3")
                f3v = f3[:].rearrange("p (a b) -> p a b", b=4)
                nc.vector.tensor_tensor(out=f3v, in0=f2v[:, :, 0:4],
                                        in1=f2v[:, :, 4:8], op=OP.max)
                f4 = fp.tile([128, NSB * 2], F16, tag="f4")
                f4v = f4[:].rearrange("p (a b) -> p a b", b=2)
                nc.vector.tensor_tensor(out=f4v, in0=f3v[:, :, 0:2],
                                        in1=f3v[:, :, 2:4], op=OP.max)
                sbm = fp.tile([128, NSB], F16, tag="sbm")
                nc.vector.tensor_tensor(out=sbm[:], in0=f4v[:, :, 0:1],
                                        in1=f4v[:, :, 1:2], op=OP.max)

                v8 = sbp.tile([128, 8], F16, tag="v8")
                nc.vector.max(v8[:], sbm[:])
                ia = sbp.tile([128, 8], U16, tag="ia")
                nc.vector.max_index(ia[:], v8[:], sbm[:])

                sbf = sbp.tile([128, 1], F32, tag="sbf")
                nc.vector.tensor_copy(sbf[:], ia[:, 0:1])
                sbu = sbp.tile([128, 1], U32, tag="sbu")
                nc.vector.tensor_copy(sbu[:], sbf[:])

                # NOTE: out must be a 2D AP — 3D outs mis-gather on HW
                cand2 = sbp.tile([128, SB * 4], F32, tag="cand")
                nc.gpsimd.indirect_dma_start(
                    out=cand2[:], out_offset=None, in_=sbt[:],
                    in_offset=bass.IndirectOffsetOnAxis(ap=sbu[:, 0:1], axis=0),
                )
                st[j]["sbf"] = sbf
                st[j]["cand2"] = cand2

            def stage_b(j):
                sbf = st[j]["sbf"]
                cand = st[j]["cand2"][:].rearrange("p (a b) -> p a b", b=4)
                # exact fp32 rescore of the 32 candidates (STT-chained)
                acc = sbp.tile([128, SB], F32, tag="acc")
                nc.vector.tensor_scalar(acc[:], cand[:, :, 0],
                                        wgv_sb[:, j, 0:1], None, OP.mult)
                acc2 = sbp.tile([128, SB], F32, tag="acc2")
                nc.vector.scalar_tensor_tensor(
                    out=acc2[:], in0=cand[:, :, 1], scalar=wgv_sb[:, j, 1:2],
                    in1=acc[:], op0=OP.mult, op1=OP.add)
                acc3 = sbp.tile([128, SB], F32, tag="acc3")
                nc.vector.scalar_tensor_tensor(
                    out=acc3[:], in0=cand[:, :, 2], scalar=wgv_sb[:, j, 2:3],
                    in1=acc2[:], op0=OP.mult, op1=OP.add)
                s2 = sbp.tile([128, SB], F32, tag="s2")
                nc.vector.scalar_tensor_tensor(
                    out=s2[:], in0=acc3[:], scalar=2.0, in1=cand[:, :, 3],
                    op0=OP.mult, op1=OP.subtract)

                r8 = sbp.tile([128, 8], F32, tag="r8")
                nc.vector.max(r8[:], s2[:])
                ip = sbp.tile([128, 8], U16, tag="ip")
                nc.vector.max_index(ip[:], r8[:], s2[:])
                ipf = sbp.tile([128, 1], F32, tag="ipf")
                nc.vector.tensor_copy(ipf[:], ip[:, 0:1])

                # idx = sb*32 + pos
                idxf = sbp.tile([128, 1], F32, tag="idxf")
                nc.vector.scalar_tensor_tensor(
                    out=idxf[:], in0=sbf[:], scalar=float(SB), in1=ipf[:],
                    op0=OP.mult, op1=OP.add)
                idxu = sbp.tile([128, 1], U32, tag="idxu")
                nc.vector.tensor_copy(idxu[:], idxf[:])

                pay = sbp.tile([128, 4], F32, tag="pay")
                nc.gpsimd.indirect_dma_start(
                    out=pay[:], out_offset=None, in_=tbl[:],
                    in_offset=bass.IndirectOffsetOnAxis(ap=idxu[:, 0:1], axis=0),
                )
                st[j]["pay"] = pay

            def stage_c(j):
                pay = st.pop(j)["pay"]
                # dots = wg . ng[idx] - pn[idx]  (STT with sum-accumulator)
                t3 = sbp.tile([128, 3], F32, tag="t3")
                dsum = sbp.tile([128, 1], F32, tag="dsum")
                nc.vector.scalar_tensor_tensor(
                    out=t3[:], in0=pay[:, 0:3], scalar=1.0,
                    in1=wgv_sb[:, j, :], op0=OP.mult, op1=OP.mult,
                    accum_out=dsum[:])
                dots = sbp.tile([128, 1], F32, tag="dots")
                nc.vector.tensor_tensor(out=dots[:], in0=dsum[:],
                                        in1=pay[:, 3:4], op=OP.subtract)

                # exp_relu(x) = max(x + 1, exp(-0.5*relu(-x)))
                rneg = sbp.tile([128, 1], F32, tag="rneg")
                nc.scalar.activation(rneg[:], dots[:], AF.Relu, scale=-1.0)
                ex = sbp.tile([128, 1], F32, tag="ex")
                nc.scalar.activation(ex[:], rneg[:], AF.Exp, scale=-0.5)
                p1 = sbp.tile([128, 1], F32, tag="p1")
                nc.scalar.activation(p1[:], dots[:], AF.Identity, bias=1.0)
                er = sbp.tile([128, 1], F32, tag="er")
                nc.vector.tensor_tensor(out=er[:], in0=p1[:], in1=ex[:],
                                        op=OP.max)
                if j == NTILES - 1:
                    erm = sbp.tile([128, 1], F32, tag="erm")
                    nc.vector.tensor_tensor(out=erm[:], in0=er[:],
                                            in1=msk_sb[:, j:j + 1],
                                            op=OP.mult)
                    er = erm
                nc.vector.tensor_tensor(out=er_sb[:, j:j + 1],
                                        in0=er_sb[:, j:j + 1], in1=er[:],
                                        op=OP.add)

            for j in range(NTILES + 2):
                if j < NTILES:
                    stage_a1(j)
                if 1 <= j < NTILES + 1:
                    stage_b(j - 1)
                if j >= 2:
                    stage_c(j - 2)
                if j < NTILES:
                    stage_a2(j)

            po = ps1.tile([1, NTILES], F32, tag="po")
            nc.tensor.matmul(out=po[:], lhsT=ones_sb[:, 0:1], rhs=er_sb[:],
                             start=True, stop=True)
            ob = sbp.tile([1, NTILES], F32, tag="ob")
            nc.vector.tensor_copy(ob[:], po[:])
            nc.sync.dma_start(out=out[:], in_=ob[:])

    nc.compile()
    return nc


def _f16_split(x32):
    hi = x32.astype(np.float16)
    lo = (x32 - hi.astype(np.float32)).astype(np.float16)
    return hi, lo


def prep_inputs(posesglobal, waypointslocal, boundary, boundarynormals):
    poses = np.asarray(posesglobal, dtype=np.float32)
    wpts = np.asarray(waypointslocal, dtype=np.float32)
    bound = np.asarray(boundary, dtype=np.float32)
    nrm = np.asarray(boundarynormals, dtype=np.float32)

    R = poses[:, :3, :3]
    t = poses[:, :3, 3]
    wg = (np.einsum("bij,btj->bti", R, wpts).astype(np.float32)
          + t[:, None, :]).astype(np.float32).reshape(-1, 3)   # [B*T, 3]

    pg = bound[:3]
    p2 = (pg[0] * pg[0] + pg[1] * pg[1] + pg[2] * pg[2]).astype(np.float32)
    pn = (pg[0] * nrm[0] + pg[1] * nrm[1] + pg[2] * nrm[2]).astype(np.float32)

    # rhs rows: per coord d -> [bh_d, bl_d, bh_d]; [ch, cl] for p2/2; ones row
    bh, bl = _f16_split(pg)
    ch, cl = _f16_split(p2 / 2.0)
    rhs = np.zeros((KS, NPAD), np.float16)
    for d in range(3):
        rhs[3 * d + 0, :N] = bh[d]
        rhs[3 * d + 1, :N] = bl[d]
        rhs[3 * d + 2, :N] = bh[d]
    rhs[9, :N] = ch
    rhs[10, :N] = cl
    rhs[9, N:] = np.float16(60000.0)   # pad columns can never win
    rhs[11, :] = np.float16(1.0)       # bias row
    # 4-way row-tiling layout: partition group 32i holds rhs quarter i
    rhs4 = np.zeros((128, NPAD // 4), np.float16)
    for i in range(4):
        rhs4[32 * i:32 * i + KS] = rhs[:, i * (NPAD // 4):(i + 1) * (NPAD // 4)]

    tb2 = np.empty((N, 4), np.float32)
    tb2[:, :3] = pg.T
    tb2[:, 3] = p2
    sbt = tb2.reshape(NSB_REAL, SB * 4)

    tbl = np.empty((N, 4), np.float32)
    tbl[:, :3] = nrm.T
    tbl[:, 3] = pn

    valid = (np.arange(WPAD) < WPC)
    msk = valid.reshape(NTILES, 128).T.astype(np.float32).copy()

    in_maps = []
    for c in range(NCORES):
        w = wg[c * WPC:(c + 1) * WPC]
        wp = np.zeros((WPAD, 3), np.float32)
        wp[:WPC] = w
        w2h = (-0.5 * (wp * wp).sum(axis=1)).astype(np.float16)  # [WPAD]
        ah, al = _f16_split(wp.T)                                # [3, WPAD]
        lhs = np.zeros((KS, WPAD), np.float16)
        for d in range(3):
            lhs[3 * d + 0] = ah[d]
            lhs[3 * d + 1] = ah[d]
            lhs[3 * d + 2] = al[d]
        lhs[9] = np.float16(-1.0)
        lhs[10] = np.float16(-1.0)
        lhs[11] = w2h
        lhs4 = np.zeros((128, WPAD), np.float16)
        for i in range(4):
            lhs4[32 * i:32 * i + KS] = lhs
        wgv = wp.reshape(NTILES, 128, 3).transpose(1, 0, 2).copy()
        in_maps.append({"lhs": lhs4, "rhs": rhs4, "wgv": wgv, "msk": msk,
                        "sbt": sbt, "tbl": tbl})
    return in_maps


_CACHE = {}


def kernel(posesglobal, waypointslocal, boundary, boundarynormals):
    if "nc" not in _CACHE:
        _CACHE["nc"] = build()
    nc = _CACHE["nc"]
    in_maps = prep_inputs(posesglobal, waypointslocal, boundary,
                          boundarynormals)
    res = run_bass_kernel_spmd(nc, in_maps, list(range(NCORES)))
    total = 0.0
    for r in res.results:
        total += float(np.asarray(r["out"], dtype=np.float64).sum())
    return np.float32(total / (B * T))
